# revision 38
# baseline (speedup 1.0000x reference)
"""ChannelGate (topk_masking) Trainium2 Bass kernel.

Strategy: pure data parallel over batch (B=32 -> 4 samples per core x 8 cores).
v2: single-pass over x. Each core loads its 4 samples' x once as resident
bf16 tiles (SWDGE cast-DMA f32->bf16), computes all stats from SBUF,
and gates in place -- no second HBM read. y is written bf16 and upcast
on host (tolerance 2e-2 >> bf16 noise).

Per sample (x layout [C=512, HW=3136] as 4 c-tiles [128, 3136] bf16):
  stats:  channel sum+max via one fused DVE tensor_tensor_reduce each
          (halves combined with add/max, accum reduces the rest);
          pixel sum via PE ones-matmul; pixel max via 3 in-place DVE
          TT-max + PE transposes + fused TTR reduces.
  topk:   channel maxes epsilon-perturbed by channel index to break
          bf16 ties, then top-256 sorted extraction via DVE max8/
          match_replace on [8, 512]; tiny MLP on PE.
  spatial: 7x7 conv via DRAM padded buffer + im2col; conv weights
          replicated to 128 columns so one PE matmul yields the
          BROADCAST spatial weight (sigmoid term + BN bias fused in as
          extra im2col rows 98/99).
  gate:   ACT sigmoid (per-partition channel scale) + DVE
          scalar_tensor_tensor out = (sig + 1) * x; DMA out bf16.
"""
import numpy as np
from contextlib import ExitStack

import concourse.bass as bass
import concourse.tile as tile
from concourse import bacc, mybir
from concourse import bass_utils
from concourse.bass_isa import ReduceOp

F32 = mybir.dt.float32
F32R = mybir.dt.float32r
BF16 = mybir.dt.bfloat16
AF = mybir.ActivationFunctionType
ALU = mybir.AluOpType
AX = mybir.AxisListType

B, C, H, W = 32, 512, 56, 56
HW = H * W            # 3136
S = 4                 # samples per core
NCORES = 8
G = 4                 # c-tiles of 128 per sample
RED = 32              # MLP hidden
NPIX_CH = 25          # ceil(3136/128) pixel chunks for transposes
CH512 = [(i * 512, min(512, HW - i * 512)) for i in range((HW + 511) // 512)]
PW = 62               # padded conv map width/height
NEG = -1.0e30
EPS_TIE = 2.0e-5      # channel-index tie-break for bf16 channel maxes


def build_program():
    nc = bacc.Bacc("TRN2", target_bir_lowering=False, debug=False,
                   num_devices=NCORES)

    x_d = nc.dram_tensor("x", [S, C, HW], F32, kind="ExternalInput")
    y_d = nc.dram_tensor("y", [S, C, HW], BF16, kind="ExternalOutput")
    w1t_d = nc.dram_tensor("w1t", [128, 4 * RED], F32, kind="ExternalInput")
    w2t_d = nc.dram_tensor("w2t", [RED, C], F32, kind="ExternalInput")
    b1_d = nc.dram_tensor("b1c", [RED, 1], F32, kind="ExternalInput")
    b2_d = nc.dram_tensor("b2c", [128, G], F32, kind="ExternalInput")
    wcr_d = nc.dram_tensor("wcrep", [100, 128], BF16, kind="ExternalInput")
    id_d = nc.dram_tensor("ident", [128, 128], F32, kind="ExternalInput")
    io_d = nc.dram_tensor("iota2", [128, 256], F32, kind="ExternalInput")
    cid_d = nc.dram_tensor("cidxeps", [128, G], F32, kind="ExternalInput")
    or_d = nc.dram_tensor("onesrow", [1, HW], BF16, kind="ExternalInput")
    pad_d = nc.dram_tensor("pad0", [S * 2 * PW * PW], BF16, kind="ExternalInput")

    with tile.TileContext(nc) as tc:
        with ExitStack() as ctx:
            build_core(ctx, tc, x_d, y_d, w1t_d, w2t_d, b1_d, b2_d,
                       wcr_d, id_d, io_d, cid_d, or_d, pad_d)
    nc.compile()
    return nc


def build_core(ctx, tc, x_d, y_d, w1t_d, w2t_d, b1_d, b2_d, wcr_d,
               id_d, io_d, cid_d, or_d, pad_d):
    nc = tc.nc

    cpool = ctx.enter_context(tc.tile_pool(name="consts", bufs=1))
    xb_pool = ctx.enter_context(tc.tile_pool(name="xb", bufs=1))
    scr_pool = ctx.enter_context(tc.tile_pool(name="scr", bufs=1))
    mx_pool = ctx.enter_context(tc.tile_pool(name="mx", bufs=3))
    row_pool = ctx.enter_context(tc.tile_pool(name="rows", bufs=1))
    ss_pool = ctx.enter_context(tc.tile_pool(name="ss", bufs=1))
    bc_pool = ctx.enter_context(tc.tile_pool(name="bc", bufs=2))
    sig_pool = ctx.enter_context(tc.tile_pool(name="sig", bufs=2))
    imt_pool = ctx.enter_context(tc.tile_pool(name="imt", bufs=2))
    sgn_pool = ctx.enter_context(tc.tile_pool(name="sgn", bufs=1))
    p_pool = ctx.enter_context(tc.tile_pool(name="ponehot", bufs=2))
    vb_pool = ctx.enter_context(tc.tile_pool(name="vb", bufs=2))

    ps_row = ctx.enter_context(tc.tile_pool(name="ps_row", bufs=2,
                                            space="PSUM"))
    ps_small = ctx.enter_context(tc.tile_pool(name="ps_small", bufs=2,
                                              space="PSUM"))
    ps_bc = ctx.enter_context(tc.tile_pool(name="ps_bc", bufs=2, space="PSUM"))
    ps_pt = ctx.enter_context(tc.tile_pool(name="ps_pt", bufs=2, space="PSUM"))

    # ---- constants / weights in SBUF ----
    ident = cpool.tile([128, 128], F32)
    nc.sync.dma_start(ident[:], id_d.ap())
    ones_bf = cpool.tile([128, 1], BF16)
    nc.vector.memset(ones_bf[:], 1.0)
    w1t = cpool.tile([128, 4 * RED], F32)
    nc.sync.dma_start(w1t[:], w1t_d.ap())
    w1etL, w1etH = w1t[:, 0:RED], w1t[:, RED:2 * RED]
    w1otL, w1otH = w1t[:, 2 * RED:3 * RED], w1t[:, 3 * RED:4 * RED]
    w2t = cpool.tile([RED, C], F32)
    nc.sync.dma_start(w2t[:], w2t_d.ap())
    b1 = cpool.tile([RED, 1], F32)
    nc.sync.dma_start(b1[:], b1_d.ap())
    b2 = cpool.tile([128, G], F32)
    nc.sync.dma_start(b2[:], b2_d.ap())
    wc_rep = cpool.tile([100, 128], BF16)
    nc.sync.dma_start(wc_rep[:], wcr_d.ap())
    cidx = cpool.tile([128, G], F32)
    nc.sync.dma_start(cidx[:], cid_d.ap())
    iota2 = cpool.tile([128, 256], F32)
    nc.sync.dma_start(iota2[:], io_d.ap())

    sc = [cpool.tile([128, 8], F32, tag=f"sc{g}", name=f"scq{g}") for g in range(G)]
    Sr = [cpool.tile([128, 8], F32, tag=f"Sr{g}", name=f"Sr{g}") for g in range(G)]
    h_sb = cpool.tile([RED, S], F32)
    sqw = [cpool.tile([128, S], F32, tag=f"sqw{g}", name=f"sqw{g}") for g in range(G)]

    ssS = ss_pool.tile([S, HW], BF16, tag="ssS")         # pixel sums (raw)
    ssM = ss_pool.tile([S, HW], BF16, tag="ssM")         # pixel maxes

    # resident x tiles (bf16)
    xb = [[xb_pool.tile([128, HW], BF16, tag=f"xb{s}_{g}", name=f"xb{s}_{g}")
           for g in range(G)] for s in range(S)]

    # ================= PHASE 1: load + stats =================
    # all loads first: keeps the in-order Pool (SWDGE) queue free of
    # compute ops so DMA issue is never stalled behind partition reduces
    for s in range(S):
        for g in range(G):
            # SWDGE cast-DMA: f32 HBM -> bf16 SBUF
            nc.gpsimd.dma_start(xb[s][g][:],
                                x_d.ap()[s, g * 128:(g + 1) * 128, :])
    for s in range(S):
        for g in range(G):
            t = xb[s][g]
            # channel sum on ACT: copy+accum (frees DVE for max trees);
            # main out is a dummy broadcast column
            scrF = scr_pool.tile([128, 1], mybir.dt.float8e4, tag="scrF")
            nc.scalar.activation(scrF.broadcast_to((128, HW)), t[:], AF.Copy,
                                 accum_out=sc[g][:, s:s + 1])
            # channel max: bf16 TT tree + short reduce
            scrB = scr_pool.tile([128, HW // 2], BF16, tag="scrB")
            nc.vector.tensor_tensor(scrB[:], t[:, 0:HW // 2], t[:, HW // 2:HW],
                                    op=ALU.max)
            scrC = scr_pool.tile([128, HW // 4], BF16, tag="scrC")
            nc.vector.tensor_tensor(scrC[:], scrB[:, 0:HW // 4],
                                    scrB[:, HW // 4:HW // 2], op=ALU.max)
            nc.vector.reduce_max(sc[g][:, 4 + s:5 + s], scrC[:], axis=AX.X)

        # pixel sums: ones.T @ x over all 4 c-tiles (bf16 PE)
        srow = row_pool.tile([1, HW], BF16, tag="row")
        for (off, wdt) in CH512:
            ps = ps_row.tile([1, 512], F32, tag='psr')
            for g in range(G):
                nc.tensor.matmul(ps[0:1, 0:wdt], ones_bf[:],
                                 xb[s][g][:, off:off + wdt],
                                 start=(g == 0), stop=(g == G - 1))
            nc.scalar.activation(srow[0:1, off:off + wdt], ps[0:1, 0:wdt],
                                 AF.Copy)
        nc.sync.dma_start(ssS[s:s + 1, :], srow[:])

        # pixel maxes: combine 4 c-tiles in place, then Q7 partition reduce
        mx = mx_pool.tile([128, HW], BF16, tag="mx")
        nc.vector.tensor_tensor(mx[:], xb[s][0][:], xb[s][1][:], op=ALU.max)
        nc.vector.tensor_tensor(mx[:], mx[:], xb[s][2][:], op=ALU.max)
        nc.vector.tensor_tensor(mx[:], mx[:], xb[s][3][:], op=ALU.max)
        nc.gpsimd.partition_all_reduce(mx[:], mx[:], 128, ReduceOp.max)
        nc.sync.dma_start(ssM[s:s + 1, :], mx[0:1, :])

    # ================= PHASE 2: rank-based topk + MLP =================
    # rank_i = #{j: v_j > v_i}; computed as S' = sum_j sign(v_i - v_j)
    # = 511 - 2*rank via ACT Sign with accum on a NEGATED PE broadcast.
    # Sorted top-256 row: t[j] = sum_i (S'_i == 511-2j) * v_i  (one fused
    # dual-scalar DVE op for the one-hot*value, PE ones-matmul to reduce).
    for g in range(G):
        # epsilon tie-break on channel maxes (bf16 values collide)
        nc.vector.tensor_scalar(out=sc[g][:, 4:8], in0=sc[g][:, 4:8],
                                scalar1=cidx[:, g:g + 1], scalar2=None,
                                op0=ALU.add)
    for r in range(8):
        # row-ify stats via DMA (bit-exact, keeps self-compare at exactly 0)
        stg = row_pool.tile([1, 512], F32, tag='stg')
        for g in range(G):
            nc.sync.dma_start(stg[0:1, g * 128:(g + 1) * 128],
                              sc[g][:, r:r + 1])
        vb = vb_pool.tile([128, 512], F32, tag='vb')
        nc.gpsimd.partition_broadcast(vb[:], stg[:], 128)
        for g in range(G):
            sgn = sgn_pool.tile([128, 512], BF16, tag='sgn')
            nc.scalar.activation(sgn[:], vb[:], AF.Sign, scale=-1.0,
                                 bias=sc[g][:, r:r + 1],
                                 accum_out=Sr[g][:, r:r + 1])
    tR = [cpool.tile([S, 256], F32, tag=f"tR{t}", name=f"tR{t}")
          for t in range(2)]
    for r in range(8):
        stat, s = r // 4, r % 4
        ptps = ps_pt.tile([1, 256], F32, tag='pt')
        for g in range(G):
            P = p_pool.tile([128, 256], BF16, tag='p')
            nc.vector.tensor_scalar(out=P[:], in0=iota2[:],
                                    scalar1=Sr[g][:, r:r + 1],
                                    scalar2=sc[g][:, r:r + 1],
                                    op0=ALU.is_equal, op1=ALU.mult)
            nc.tensor.matmul(ptps[:], ones_bf[:], P[:],
                             start=(g == 0), stop=(g == G - 1))
        trow = row_pool.tile([1, 256], F32, tag=f'trow{r % 2}',
                             name=f'trow{r}')
        nc.vector.tensor_copy(trow[:], ptps[:])
        nc.sync.dma_start(tR[stat][s:s + 1, :], trow[:])
    # transpose sorted rows into MLP rhs layout [128, 4] (k-half x sample)
    tcols = [cpool.tile([128, S], F32, tag=f"tc{q}", name=f"tc{q}")
             for q in range(4)]
    for q in range(4):
        stat, half = q // 2, q % 2
        pst = ps_small.tile([128, S], F32, tag='pss')
        nc.tensor.transpose(pst[:], tR[stat][:, 128 * half:128 * half + 128],
                            ident[0:4, 0:4])
        nc.vector.tensor_copy(tcols[q][:], pst[:])
    # h = relu(W1e @ t1 + W1o @ t2 + b1)  (weights pre-transposed, 1/HW
    # folded into the sum-side weights host-side)
    psh = ps_small.tile([RED, S], F32, tag='pss')
    wmat = [w1etL, w1etH, w1otL, w1otH]
    for q in range(4):
        nc.tensor.matmul(psh[:], wmat[q], tcols[q][:],
                         start=(q == 0), stop=(q == 3))
    nc.scalar.activation(h_sb[:], psh[:], AF.Relu, bias=b1[:])
    # mlp_out per c-tile; squeeze_weight = relu(mlp_out + b2 + sigmoid(sc1*sc2))
    for g in range(G):
        psm = ps_small.tile([128, S], F32, tag='pss')
        nc.tensor.matmul(psm[:], w2t[:, g * 128:(g + 1) * 128], h_sb[:],
                         start=True, stop=True)
        prod = cpool.tile([128, S], F32, tag=f"prod{g}")
        nc.vector.tensor_tensor(prod[:], sc[g][:, 0:4], sc[g][:, 4:8],
                                op=ALU.mult)
        sigp = cpool.tile([128, S], F32, tag=f"sigp{g}")
        nc.scalar.activation(sigp[:], prod[:], AF.Sigmoid, scale=1.0 / HW)
        nc.vector.tensor_tensor(sigp[:], sigp[:], psm[:], op=ALU.add)
        nc.scalar.activation(sqw[g][:], sigp[:], AF.Relu, bias=b2[:, g:g + 1])

    # ================= SPATIAL PATH =================
    prodS = ss_pool.tile([S, HW], BF16, tag="prodS")
    nc.vector.tensor_tensor(prodS[:], ssS[:], ssM[:], op=ALU.mult)
    nc.scalar.activation(prodS[:], prodS[:], AF.Sigmoid, scale=1.0 / C)
    # conv: padded interiors -> im2col -> fused conv+sig+bias+broadcast
    for s in range(S):
        for ci, src2 in ((0, ssS), (1, ssM)):
            base = ((s * 2 + ci) * PW + 3) * PW + 3
            dst = bass.AP(pad_d, base, [[PW, H], [1, W]])
            nc.gpsimd.dma_start(dst,
                                src2[s:s + 1, :].rearrange("p (h w) -> p h w",
                                                           h=H))
    for s in range(S):
        imt = imt_pool.tile([100, HW], BF16)
        for ci in range(2):
            for kh in range(7):
                base = ((s * 2 + ci) * PW + kh) * PW
                src = bass.AP(pad_d, base, [[1, 7], [PW, H], [1, W]])
                p0 = ci * 49 + kh * 7
                nc.sync.dma_start(imt[p0:p0 + 7, :], src)
        # row 98: sigmoid(ss1*ss2) weight 1.0; row 99: ones, weight k2 (BN)
        nc.sync.dma_start(imt[98:99, :], prodS[s:s + 1, :])
        nc.sync.dma_start(imt[99:100, :], or_d.ap())
        # fused conv + broadcast: wc_rep [100,128] makes psum [128,wdt] be
        # the spatial weight replicated across all 128 partitions
        bcS = bc_pool.tile([128, HW], BF16, tag="bcS", name=f"bcS{s}")
        for (off, wdt) in CH512:
            psb = ps_bc.tile([128, 512], F32, tag='psb')
            nc.tensor.matmul(psb[:, 0:wdt], wc_rep[:],
                             imt[:, off:off + wdt], start=True, stop=True)
            nc.scalar.activation(bcS[:, off:off + wdt], psb[:, 0:wdt],
                                 AF.Copy)

        # ================= PHASE 3: gate =================
        for g in range(G):
            sg = sig_pool.tile([128, HW], BF16, tag="sg")
            nc.scalar.activation(sg[:], bcS[:], AF.Sigmoid,
                                 scale=sqw[g][:, s:s + 1])
            # (sig+1)*x as 4x-mode ts + 2x-mode TT (stt runs 1x only)
            nc.vector.tensor_scalar(out=sg[:], in0=sg[:], scalar1=1.0,
                                    scalar2=None, op0=ALU.add)
            yt = mx_pool.tile([128, HW], BF16, tag="mx", name=f"yt{s}_{g}")
            nc.vector.tensor_tensor(yt[:], sg[:], xb[s][g][:], op=ALU.mult)
            # y-out on the ACT HWDGE queue: keeps the Sync queue free for
            # the next sample's im2col loads (in-order queues)
            nc.scalar.dma_start(y_d.ap()[s, g * 128:(g + 1) * 128, :], yt[:])


_NC_CACHE = {}


def _get_program():
    if "nc" not in _NC_CACHE:
        _NC_CACHE["nc"] = build_program()
    return _NC_CACHE["nc"]


def _host_params(w1, b1, w2, b2, conv_w, bn_gamma, bn_beta, bn_mean, bn_var):
    import ml_dtypes
    w1 = np.asarray(w1, np.float32)
    w2 = np.asarray(w2, np.float32)
    b1 = np.asarray(b1, np.float32)
    b2 = np.asarray(b2, np.float32)
    conv_w = np.asarray(conv_w, np.float32)

    w1et = w1[:, 0::2].T / HW                           # [256, 32], 1/HW fold
    w1ot = w1[:, 1::2].T                                # [256, 32]
    w1t = np.ascontiguousarray(np.concatenate(
        [w1et[0:128], w1et[128:256], w1ot[0:128], w1ot[128:256]],
        axis=1)).astype(np.float32)                     # [128, 128]
    w2t = np.ascontiguousarray(w2.T)                    # [32, 512]
    b1c = b1.reshape(RED, 1).copy()
    b2c = np.ascontiguousarray(b2.reshape(G, 128).T)    # [128, G]

    bn_scale = float(bn_gamma[0]) / np.sqrt(float(bn_var[0]) + 1e-5)
    k2 = float(bn_beta[0]) - float(bn_mean[0]) * bn_scale
    wcf = conv_w[0].astype(np.float64) * bn_scale       # [2, 7, 7]
    wcf = wcf.copy()
    wcf[0] /= C                                         # mean channel fold
    # conv taps + sigmoid row (1.0) + BN-bias row (k2), replicated 128 wide
    wc100 = np.concatenate([wcf.reshape(98), [1.0], [k2]]).astype(np.float32)
    wcrep = np.repeat(wc100[:, None], 128, axis=1).astype(ml_dtypes.bfloat16)

    ident = np.eye(128, dtype=np.float32)
    cidxeps = (EPS_TIE * (np.arange(128)[:, None]
                          + 128.0 * np.arange(G)[None, :])).astype(np.float32)
    iota2 = np.tile((511.0 - 2.0 * np.arange(256, dtype=np.float32)),
                    (128, 1)).astype(np.float32)
    onesrow = np.ones((1, HW), ml_dtypes.bfloat16)
    pad0 = np.zeros(S * 2 * PW * PW, ml_dtypes.bfloat16)
    return dict(w1t=w1t, w2t=w2t, b1c=b1c, b2c=b2c, wcrep=wcrep,
                ident=ident, iota2=iota2, cidxeps=cidxeps,
                onesrow=onesrow, pad0=pad0)


def kernel(x, w1, b1, w2, b2, conv_w, bn_gamma, bn_beta, bn_mean, bn_var):
    x = np.asarray(x, np.float32)
    params = _host_params(w1, b1, w2, b2, conv_w,
                          bn_gamma, bn_beta, bn_mean, bn_var)
    nc = _get_program()

    xr = x.reshape(B, C, HW)
    in_maps = []
    for k in range(NCORES):
        m = {"x": np.ascontiguousarray(xr[k * S:(k + 1) * S])}
        m.update(params)
        in_maps.append(m)

    res = bass_utils.run_bass_kernel_spmd(nc, in_maps,
                                          core_ids=list(range(NCORES)))
    out = np.concatenate([np.asarray(res.results[k]["y"])
                          for k in range(NCORES)], axis=0)
    return out.reshape(B, C, H, W).astype(np.float32)


# revision 42
# speedup vs baseline: 1.0382x; 1.0382x over previous
"""ChannelGate (topk_masking) Trainium2 Bass kernel.

Strategy: pure data parallel over batch (B=32 -> 4 samples per core x 8 cores).
v2: single-pass over x. Each core loads its 4 samples' x once as resident
bf16 tiles (SWDGE cast-DMA f32->bf16), computes all stats from SBUF,
and gates in place -- no second HBM read. y is written bf16 and upcast
on host (tolerance 2e-2 >> bf16 noise).

Per sample (x layout [C=512, HW=3136] as 4 c-tiles [128, 3136] bf16):
  stats:  channel sum+max via one fused DVE tensor_tensor_reduce each
          (halves combined with add/max, accum reduces the rest);
          pixel sum via PE ones-matmul; pixel max via 3 in-place DVE
          TT-max + PE transposes + fused TTR reduces.
  topk:   channel maxes epsilon-perturbed by channel index to break
          bf16 ties, then top-256 sorted extraction via DVE max8/
          match_replace on [8, 512]; tiny MLP on PE.
  spatial: 7x7 conv via DRAM padded buffer + im2col; conv weights
          replicated to 128 columns so one PE matmul yields the
          BROADCAST spatial weight (sigmoid term + BN bias fused in as
          extra im2col rows 98/99).
  gate:   ACT sigmoid (per-partition channel scale) + DVE
          scalar_tensor_tensor out = (sig + 1) * x; DMA out bf16.
"""
import numpy as np
from contextlib import ExitStack

import concourse.bass as bass
import concourse.tile as tile
from concourse import bacc, mybir
from concourse import bass_utils
from concourse.bass_isa import ReduceOp

F32 = mybir.dt.float32
F32R = mybir.dt.float32r
BF16 = mybir.dt.bfloat16
AF = mybir.ActivationFunctionType
ALU = mybir.AluOpType
AX = mybir.AxisListType

B, C, H, W = 32, 512, 56, 56
HW = H * W            # 3136
S = 4                 # samples per core
NCORES = 8
G = 4                 # c-tiles of 128 per sample
RED = 32              # MLP hidden
NPIX_CH = 25          # ceil(3136/128) pixel chunks for transposes
CH512 = [(i * 512, min(512, HW - i * 512)) for i in range((HW + 511) // 512)]
PW = 62               # padded conv map width/height
NEG = -1.0e30
EPS_TIE = 2.0e-5      # channel-index tie-break for bf16 channel maxes


def build_program():
    nc = bacc.Bacc("TRN2", target_bir_lowering=False, debug=False,
                   num_devices=NCORES)

    x_d = nc.dram_tensor("x", [S, C, HW], F32, kind="ExternalInput")
    y_d = nc.dram_tensor("y", [S, C, HW], BF16, kind="ExternalOutput")
    w1t_d = nc.dram_tensor("w1t", [128, 4 * RED], F32, kind="ExternalInput")
    w2t_d = nc.dram_tensor("w2t", [RED, C], F32, kind="ExternalInput")
    b1_d = nc.dram_tensor("b1c", [RED, 1], F32, kind="ExternalInput")
    b2_d = nc.dram_tensor("b2c", [128, G], F32, kind="ExternalInput")
    wcr_d = nc.dram_tensor("wcrep", [100, 128], BF16, kind="ExternalInput")
    id_d = nc.dram_tensor("ident", [128, 128], F32, kind="ExternalInput")
    io_d = nc.dram_tensor("iota2", [128, 256], F32, kind="ExternalInput")
    cid_d = nc.dram_tensor("cidxeps", [128, G], F32, kind="ExternalInput")
    or_d = nc.dram_tensor("onesrow", [1, HW], BF16, kind="ExternalInput")
    pad_d = nc.dram_tensor("pad0", [S * 2 * PW * PW], BF16, kind="ExternalInput")

    with tile.TileContext(nc) as tc:
        with ExitStack() as ctx:
            build_core(ctx, tc, x_d, y_d, w1t_d, w2t_d, b1_d, b2_d,
                       wcr_d, id_d, io_d, cid_d, or_d, pad_d)
    nc.compile()
    return nc


def build_core(ctx, tc, x_d, y_d, w1t_d, w2t_d, b1_d, b2_d, wcr_d,
               id_d, io_d, cid_d, or_d, pad_d):
    nc = tc.nc

    cpool = ctx.enter_context(tc.tile_pool(name="consts", bufs=1))
    xb_pool = ctx.enter_context(tc.tile_pool(name="xb", bufs=1))
    scr_pool = ctx.enter_context(tc.tile_pool(name="scr", bufs=1))
    mx_pool = ctx.enter_context(tc.tile_pool(name="mx", bufs=2))
    row_pool = ctx.enter_context(tc.tile_pool(name="rows", bufs=1))
    ss_pool = ctx.enter_context(tc.tile_pool(name="ss", bufs=1))
    bc_pool = ctx.enter_context(tc.tile_pool(name="bc", bufs=1))
    sig_pool = ctx.enter_context(tc.tile_pool(name="sig", bufs=3))
    imt_pool = ctx.enter_context(tc.tile_pool(name="imt", bufs=2))
    sgn_pool = ctx.enter_context(tc.tile_pool(name="sgn", bufs=1))
    p_pool = ctx.enter_context(tc.tile_pool(name="ponehot", bufs=1))
    vb_pool = ctx.enter_context(tc.tile_pool(name="vb", bufs=1))

    ps_row = ctx.enter_context(tc.tile_pool(name="ps_row", bufs=2,
                                            space="PSUM"))
    ps_small = ctx.enter_context(tc.tile_pool(name="ps_small", bufs=2,
                                              space="PSUM"))
    ps_bc = ctx.enter_context(tc.tile_pool(name="ps_bc", bufs=2, space="PSUM"))
    ps_pt = ctx.enter_context(tc.tile_pool(name="ps_pt", bufs=2, space="PSUM"))

    # ---- constants / weights in SBUF ----
    ident = cpool.tile([128, 128], F32)
    nc.sync.dma_start(ident[:], id_d.ap())
    ones_bf = cpool.tile([128, 1], BF16)
    nc.vector.memset(ones_bf[:], 1.0)
    w1t = cpool.tile([128, 4 * RED], F32)
    nc.sync.dma_start(w1t[:], w1t_d.ap())
    w1etL, w1etH = w1t[:, 0:RED], w1t[:, RED:2 * RED]
    w1otL, w1otH = w1t[:, 2 * RED:3 * RED], w1t[:, 3 * RED:4 * RED]
    w2t = cpool.tile([RED, C], F32)
    nc.sync.dma_start(w2t[:], w2t_d.ap())
    b1 = cpool.tile([RED, 1], F32)
    nc.sync.dma_start(b1[:], b1_d.ap())
    b2 = cpool.tile([128, G], F32)
    nc.sync.dma_start(b2[:], b2_d.ap())
    wc_rep = cpool.tile([100, 128], BF16)
    nc.sync.dma_start(wc_rep[:], wcr_d.ap())
    cidx = cpool.tile([128, G], F32)
    nc.sync.dma_start(cidx[:], cid_d.ap())
    iota2 = cpool.tile([128, 256], F32)
    nc.sync.dma_start(iota2[:], io_d.ap())

    sc = [cpool.tile([128, 8], F32, tag=f"sc{g}", name=f"scq{g}") for g in range(G)]
    Sr = [cpool.tile([128, 8], F32, tag=f"Sr{g}", name=f"Sr{g}") for g in range(G)]
    h_sb = cpool.tile([RED, S], F32)
    sqw = [cpool.tile([128, S], F32, tag=f"sqw{g}", name=f"sqw{g}") for g in range(G)]

    ssS = ss_pool.tile([S, HW], BF16, tag="ssS")         # pixel sums (raw)
    ssM = ss_pool.tile([S, HW], BF16, tag="ssM")         # pixel maxes
    bcS = [bc_pool.tile([128, HW], BF16, tag=f"bcS{s}", name=f"bcS{s}")
           for s in range(S)]                            # spatial weights

    # resident x tiles (bf16)
    xb = [[xb_pool.tile([128, HW], BF16, tag=f"xb{s}_{g}", name=f"xb{s}_{g}")
           for g in range(G)] for s in range(S)]

    # ================= PHASE 1: load + stats =================
    # all loads first: keeps the in-order Pool (SWDGE) queue free of
    # compute ops so DMA issue is never stalled behind partition reduces
    for s in range(S):
        for g in range(G):
            # SWDGE cast-DMA: f32 HBM -> bf16 SBUF
            nc.gpsimd.dma_start(xb[s][g][:],
                                x_d.ap()[s, g * 128:(g + 1) * 128, :])
    for s in range(S):
        for g in range(G):
            t = xb[s][g]
            # channel sum on ACT: copy+accum (frees DVE for max trees);
            # main out is a dummy broadcast column
            scrF = scr_pool.tile([128, 1], mybir.dt.float8e4, tag="scrF")
            nc.scalar.activation(scrF.broadcast_to((128, HW)), t[:], AF.Copy,
                                 accum_out=sc[g][:, s:s + 1])
            # channel max: bf16 TT tree + short reduce
            scrB = scr_pool.tile([128, HW // 2], BF16, tag="scrB")
            nc.vector.tensor_tensor(scrB[:], t[:, 0:HW // 2], t[:, HW // 2:HW],
                                    op=ALU.max)
            scrC = scr_pool.tile([128, HW // 4], BF16, tag="scrC")
            nc.vector.tensor_tensor(scrC[:], scrB[:, 0:HW // 4],
                                    scrB[:, HW // 4:HW // 2], op=ALU.max)
            nc.vector.reduce_max(sc[g][:, 4 + s:5 + s], scrC[:], axis=AX.X)

        # pixel sums: ones.T @ x over all 4 c-tiles (bf16 PE)
        srow = row_pool.tile([1, HW], BF16, tag="row")
        for (off, wdt) in CH512:
            ps = ps_row.tile([1, 512], F32, tag='psr')
            for g in range(G):
                nc.tensor.matmul(ps[0:1, 0:wdt], ones_bf[:],
                                 xb[s][g][:, off:off + wdt],
                                 start=(g == 0), stop=(g == G - 1))
            nc.scalar.activation(srow[0:1, off:off + wdt], ps[0:1, 0:wdt],
                                 AF.Copy)
        nc.sync.dma_start(ssS[s:s + 1, :], srow[:])

        # pixel maxes: combine 4 c-tiles in place, then Q7 partition reduce
        mx = mx_pool.tile([128, HW], BF16, tag="mx")
        nc.vector.tensor_tensor(mx[:], xb[s][0][:], xb[s][1][:], op=ALU.max)
        nc.vector.tensor_tensor(mx[:], mx[:], xb[s][2][:], op=ALU.max)
        nc.vector.tensor_tensor(mx[:], mx[:], xb[s][3][:], op=ALU.max)
        nc.gpsimd.partition_all_reduce(mx[:], mx[:], 128, ReduceOp.max)
        nc.sync.dma_start(ssM[s:s + 1, :], mx[0:1, :])

        # pad interiors for the conv (reads ssS/ssM rows of this sample)
        for ci, src2 in ((0, ssS), (1, ssM)):
            base = ((s * 2 + ci) * PW + 3) * PW + 3
            dst = bass.AP(pad_d, base, [[PW, H], [1, W]])
            nc.gpsimd.dma_start(dst,
                                src2[s:s + 1, :].rearrange("p (h w) -> p h w",
                                                           h=H))
        if s % 2 == 1:
            # pair sigmoid(ss1*ss2) in place over the ssS rows (pads of the
            # pair already read them), then the pair's spatial weights
            p0 = s - 1
            nc.vector.tensor_tensor(ssS[0:s + 1, :], ssS[0:s + 1, :],
                                    ssM[0:s + 1, :], op=ALU.mult)
            nc.scalar.activation(ssS[0:s + 1, :], ssS[0:s + 1, :],
                                 AF.Sigmoid, scale=1.0 / C)
            for s2 in (p0, s):
                imt = imt_pool.tile([100, HW], BF16)
                for ci in range(2):
                    for kh in range(7):
                        base = ((s2 * 2 + ci) * PW + kh) * PW
                        src = bass.AP(pad_d, base, [[1, 7], [PW, H], [1, W]])
                        p1 = ci * 49 + kh * 7
                        nc.sync.dma_start(imt[p1:p1 + 7, :], src)
                # row 98: sigmoid row (weight 1.0); row 99: ones (weight k2)
                nc.sync.dma_start(imt[98:99, :], ssS[s2:s2 + 1, :])
                nc.sync.dma_start(imt[99:100, :], or_d.ap())
                # fused conv + broadcast: psum [128,wdt] = spatial weight
                # replicated across partitions
                for (off, wdt) in CH512:
                    psb = ps_bc.tile([128, 512], F32, tag='psb')
                    nc.tensor.matmul(psb[:, 0:wdt], wc_rep[:],
                                     imt[:, off:off + wdt],
                                     start=True, stop=True)
                    nc.scalar.activation(bcS[s2][:, off:off + wdt],
                                         psb[:, 0:wdt], AF.Copy)

    # ================= PHASE 2: rank-based topk + MLP =================
    # rank_i = #{j: v_j > v_i}; computed as S' = sum_j sign(v_i - v_j)
    # = 511 - 2*rank via ACT Sign with accum on a NEGATED PE broadcast.
    # Sorted top-256 row: t[j] = sum_i (S'_i == 511-2j) * v_i  (one fused
    # dual-scalar DVE op for the one-hot*value, PE ones-matmul to reduce).
    for g in range(G):
        # epsilon tie-break on channel maxes (bf16 values collide)
        nc.vector.tensor_scalar(out=sc[g][:, 4:8], in0=sc[g][:, 4:8],
                                scalar1=cidx[:, g:g + 1], scalar2=None,
                                op0=ALU.add)
    for r in range(8):
        # row-ify stats via DMA (bit-exact, keeps self-compare at exactly 0)
        stg = row_pool.tile([1, 512], F32, tag='stg')
        for g in range(G):
            nc.sync.dma_start(stg[0:1, g * 128:(g + 1) * 128],
                              sc[g][:, r:r + 1])
        vb = vb_pool.tile([128, 512], F32, tag='vb')
        nc.gpsimd.partition_broadcast(vb[:], stg[:], 128)
        for g in range(G):
            sgn = sgn_pool.tile([128, 512], BF16, tag='sgn')
            nc.scalar.activation(sgn[:], vb[:], AF.Sign, scale=-1.0,
                                 bias=sc[g][:, r:r + 1],
                                 accum_out=Sr[g][:, r:r + 1])
    tR = [cpool.tile([S, 256], F32, tag=f"tR{t}", name=f"tR{t}")
          for t in range(2)]
    for r in range(8):
        stat, s = r // 4, r % 4
        ptps = ps_pt.tile([1, 256], F32, tag='pt')
        for g in range(G):
            P = p_pool.tile([128, 256], BF16, tag='p')
            nc.vector.tensor_scalar(out=P[:], in0=iota2[:],
                                    scalar1=Sr[g][:, r:r + 1],
                                    scalar2=sc[g][:, r:r + 1],
                                    op0=ALU.is_equal, op1=ALU.mult)
            nc.tensor.matmul(ptps[:], ones_bf[:], P[:],
                             start=(g == 0), stop=(g == G - 1))
        trow = row_pool.tile([1, 256], F32, tag=f'trow{r % 2}',
                             name=f'trow{r}')
        nc.vector.tensor_copy(trow[:], ptps[:])
        nc.sync.dma_start(tR[stat][s:s + 1, :], trow[:])
    # transpose sorted rows into MLP rhs layout [128, 4] (k-half x sample)
    tcols = [cpool.tile([128, S], F32, tag=f"tc{q}", name=f"tc{q}")
             for q in range(4)]
    for q in range(4):
        stat, half = q // 2, q % 2
        pst = ps_small.tile([128, S], F32, tag='pss')
        nc.tensor.transpose(pst[:], tR[stat][:, 128 * half:128 * half + 128],
                            ident[0:4, 0:4])
        nc.vector.tensor_copy(tcols[q][:], pst[:])
    # h = relu(W1e @ t1 + W1o @ t2 + b1)  (weights pre-transposed, 1/HW
    # folded into the sum-side weights host-side)
    psh = ps_small.tile([RED, S], F32, tag='pss')
    wmat = [w1etL, w1etH, w1otL, w1otH]
    for q in range(4):
        nc.tensor.matmul(psh[:], wmat[q], tcols[q][:],
                         start=(q == 0), stop=(q == 3))
    nc.scalar.activation(h_sb[:], psh[:], AF.Relu, bias=b1[:])
    # mlp_out per c-tile; squeeze_weight = relu(mlp_out + b2 + sigmoid(sc1*sc2))
    for g in range(G):
        psm = ps_small.tile([128, S], F32, tag='pss')
        nc.tensor.matmul(psm[:], w2t[:, g * 128:(g + 1) * 128], h_sb[:],
                         start=True, stop=True)
        prod = cpool.tile([128, S], F32, tag=f"prod{g}")
        nc.vector.tensor_tensor(prod[:], sc[g][:, 0:4], sc[g][:, 4:8],
                                op=ALU.mult)
        sigp = cpool.tile([128, S], F32, tag=f"sigp{g}")
        nc.scalar.activation(sigp[:], prod[:], AF.Sigmoid, scale=1.0 / HW)
        nc.vector.tensor_tensor(sigp[:], sigp[:], psm[:], op=ALU.add)
        nc.scalar.activation(sqw[g][:], sigp[:], AF.Relu, bias=b2[:, g:g + 1])

    # ================= PHASE 3: gate (all deps prebuilt) =================
    for s in range(S):
        for g in range(G):
            sg = sig_pool.tile([128, HW], BF16, tag="sg")
            nc.scalar.activation(sg[:], bcS[s][:], AF.Sigmoid,
                                 scale=sqw[g][:, s:s + 1])
            # (sig+1)*x as 4x-mode ts + 2x-mode TT, all in place
            nc.vector.tensor_scalar(out=sg[:], in0=sg[:], scalar1=1.0,
                                    scalar2=None, op0=ALU.add)
            nc.vector.tensor_tensor(sg[:], sg[:], xb[s][g][:], op=ALU.mult)
            # y-out on the ACT HWDGE queue: keeps the Sync queue free
            nc.scalar.dma_start(y_d.ap()[s, g * 128:(g + 1) * 128, :], sg[:])


_NC_CACHE = {}


def _get_program():
    if "nc" not in _NC_CACHE:
        _NC_CACHE["nc"] = build_program()
    return _NC_CACHE["nc"]


def _host_params(w1, b1, w2, b2, conv_w, bn_gamma, bn_beta, bn_mean, bn_var):
    import ml_dtypes
    w1 = np.asarray(w1, np.float32)
    w2 = np.asarray(w2, np.float32)
    b1 = np.asarray(b1, np.float32)
    b2 = np.asarray(b2, np.float32)
    conv_w = np.asarray(conv_w, np.float32)

    w1et = w1[:, 0::2].T / HW                           # [256, 32], 1/HW fold
    w1ot = w1[:, 1::2].T                                # [256, 32]
    w1t = np.ascontiguousarray(np.concatenate(
        [w1et[0:128], w1et[128:256], w1ot[0:128], w1ot[128:256]],
        axis=1)).astype(np.float32)                     # [128, 128]
    w2t = np.ascontiguousarray(w2.T)                    # [32, 512]
    b1c = b1.reshape(RED, 1).copy()
    b2c = np.ascontiguousarray(b2.reshape(G, 128).T)    # [128, G]

    bn_scale = float(bn_gamma[0]) / np.sqrt(float(bn_var[0]) + 1e-5)
    k2 = float(bn_beta[0]) - float(bn_mean[0]) * bn_scale
    wcf = conv_w[0].astype(np.float64) * bn_scale       # [2, 7, 7]
    wcf = wcf.copy()
    wcf[0] /= C                                         # mean channel fold
    # conv taps + sigmoid row (1.0) + BN-bias row (k2), replicated 128 wide
    wc100 = np.concatenate([wcf.reshape(98), [1.0], [k2]]).astype(np.float32)
    wcrep = np.repeat(wc100[:, None], 128, axis=1).astype(ml_dtypes.bfloat16)

    ident = np.eye(128, dtype=np.float32)
    cidxeps = (EPS_TIE * (np.arange(128)[:, None]
                          + 128.0 * np.arange(G)[None, :])).astype(np.float32)
    iota2 = np.tile((511.0 - 2.0 * np.arange(256, dtype=np.float32)),
                    (128, 1)).astype(np.float32)
    onesrow = np.ones((1, HW), ml_dtypes.bfloat16)
    pad0 = np.zeros(S * 2 * PW * PW, ml_dtypes.bfloat16)
    return dict(w1t=w1t, w2t=w2t, b1c=b1c, b2c=b2c, wcrep=wcrep,
                ident=ident, iota2=iota2, cidxeps=cidxeps,
                onesrow=onesrow, pad0=pad0)


def kernel(x, w1, b1, w2, b2, conv_w, bn_gamma, bn_beta, bn_mean, bn_var):
    x = np.asarray(x, np.float32)
    params = _host_params(w1, b1, w2, b2, conv_w,
                          bn_gamma, bn_beta, bn_mean, bn_var)
    nc = _get_program()

    xr = x.reshape(B, C, HW)
    in_maps = []
    for k in range(NCORES):
        m = {"x": np.ascontiguousarray(xr[k * S:(k + 1) * S])}
        m.update(params)
        in_maps.append(m)

    res = bass_utils.run_bass_kernel_spmd(nc, in_maps,
                                          core_ids=list(range(NCORES)))
    out = np.concatenate([np.asarray(res.results[k]["y"])
                          for k in range(NCORES)], axis=0)
    return out.reshape(B, C, H, W).astype(np.float32)
